# revision 1
# baseline (speedup 1.0000x reference)
"""MiniTransformerLayer on 8 Trainium2 NeuronCores.

Sharding (single kernel launch, 2 collectives, no all-reduce):
  - tokens t = b*S + s flattened to [4096]; core c owns tokens [512c, 512(c+1))
    and heads {2c, 2c+1} (for both batches).
  - LN1 computed on own token shard (activations kept transposed [hidden, token]),
    AllGather -> full h^T on every core.
  - qkv column-sharded by head. q,k produced feature-major [d, t] (with an
    even/odd d-permutation so RoPE needs no partition swaps), v token-major [t, d].
  - attention per (batch, head): scores computed transposed (s^T[k,q] = k^T.T @ q^T),
    exp on ScalarE (constant -3 bias instead of row-max; cancels in normalization),
    denominator = ones-vector matmul over a DVE-folded chunk accumulator,
    attn@V contracts k directly with p^T as the moving operand -> out [d, q].
  - AllToAll converts head-sharded attn output to token-sharded full-feature.
  - out_proj / MLP computed data-parallel on own 512 tokens with replicated
    (streamed) weights. Residual path in fp32; matmul operands fp16.
"""

import sys

sys.path.insert(0, "/opt/trn_rl_repo")

import numpy as np

import concourse.bass as bass
import concourse.bacc as bacc
import concourse.tile as tile
import concourse.mybir as mybir
from concourse import bass_utils

F16 = mybir.dt.float16
F32 = mybir.dt.float32
AF = mybir.ActivationFunctionType

NCORES = 8
B, S, HID, HEADS, D, FFN = 2, 2048, 2048, 16, 128, 4096
TOK = B * S            # 4096 flat tokens
TPC = TOK // NCORES    # 512 tokens per core
HC = HID // 128        # 16 hidden chunks
FFC = FFN // 128       # 32 ffn chunks
NH = HEADS // NCORES   # 2 heads per core
SCALE = 1.0 / float(np.sqrt(D))
EXP_BIAS = -3.0
EPS = 1e-5

_CACHE = {}


def _emit(nc, single_core=False):
    xT = nc.dram_tensor("xT", [HID, TPC], F32, kind="ExternalInput")
    wq = nc.dram_tensor("wq", [128, HC * NH * 128], F16, kind="ExternalInput")
    wk = nc.dram_tensor("wk", [128, HC * NH * 128], F16, kind="ExternalInput")
    wv = nc.dram_tensor("wv", [128, HC * NH * 128], F16, kind="ExternalInput")
    wo = nc.dram_tensor("wo", [HC * 128, HC * 128], F16, kind="ExternalInput")
    wf1 = nc.dram_tensor("wf1", [FFC * 128, HC * 128], F16, kind="ExternalInput")
    wf2 = nc.dram_tensor("wf2", [HC * 128, FFC * 128], F16, kind="ExternalInput")
    g1 = nc.dram_tensor("g1", [128, HC], F32, kind="ExternalInput")
    b1 = nc.dram_tensor("b1", [128, HC], F32, kind="ExternalInput")
    g2 = nc.dram_tensor("g2", [128, HC], F32, kind="ExternalInput")
    b2 = nc.dram_tensor("b2", [128, HC], F32, kind="ExternalInput")
    ropeC = nc.dram_tensor("ropeC", [128, TOK], F16, kind="ExternalInput")
    ropeS = nc.dram_tensor("ropeS", [128, TOK], F16, kind="ExternalInput")
    outT = nc.dram_tensor("outT", [HID, TPC], F32, kind="ExternalOutput")

    rg = [list(range(NCORES))]
    MULT, ADD = mybir.AluOpType.mult, mybir.AluOpType.add

    with tile.TileContext(nc) as tc:
        with (
            tc.tile_pool(name="const", bufs=1) as const,
            tc.tile_pool(name="dram", bufs=1, space="DRAM") as dram,
        ):
            ones_col = const.tile([128, 1], F32, tag="onc")
            nc.vector.memset(ones_col[:], 1.0)
            ones_col16 = const.tile([128, 1], F16, tag="onc16")
            nc.vector.memset(ones_col16[:], 1.0)
            ones_row = const.tile([1, 128], F32, tag="onr")
            nc.vector.memset(ones_row[:], 1.0)
            eps_b = const.tile([1, 1], F32, tag="epsb")
            nc.vector.memset(eps_b[:], EPS)
            zero1_b = const.tile([1, 1], F32, tag="z1b")
            nc.vector.memset(zero1_b[:], 0.0)
            zero_b = const.tile([128, 1], F32, tag="zb")
            nc.vector.memset(zero_b[:], 0.0)
            expb_b = const.tile([128, 1], F32, tag="expb")
            nc.vector.memset(expb_b[:], EXP_BIAS)
            g1_sb = const.tile([128, HC], F32, tag="g1")
            b1_sb = const.tile([128, HC], F32, tag="b1")
            g2_sb = const.tile([128, HC], F32, tag="g2")
            b2_sb = const.tile([128, HC], F32, tag="b2")
            nc.scalar.dma_start(g1_sb[:], g1[:])
            nc.scalar.dma_start(b1_sb[:], b1[:])
            nc.scalar.dma_start(g2_sb[:], g2[:])
            nc.scalar.dma_start(b2_sb[:], b2[:])

            ag_in_a = dram.tile([HID // 2, TPC], F16)
            ag_in_b = dram.tile([HID // 2, TPC], F16)
            a2a_in_m = [dram.tile([NCORES * 128, TPC], F16, name=f"a2ai{m}")
                        for m in range(NH)]
            a2a_out_m = [dram.tile([NCORES * 128, TPC], F16,
                                   name=f"a2ao{m}") for m in range(NH)]
            if single_core:
                ag_out_a = dram.tile([NCORES * HID // 2, TPC], F16)
                ag_out_b = dram.tile([NCORES * HID // 2, TPC], F16)
            else:
                ag_out_a = nc.dram_tensor(
                    "ag_out_a_sh", [NCORES * HID // 2, TPC], F16,
                    addr_space="Shared").ap()
                ag_out_b = nc.dram_tensor(
                    "ag_out_b_sh", [NCORES * HID // 2, TPC], F16,
                    addr_space="Shared").ap()

            def layernorm(get_src, put_dst, gg, bb, lnp, psst, psbc):
                # h = (x - mu) * rstd * g + b, contraction over partitions via
                # ones-matmuls; per-token coeffs broadcast via K=1 matmuls.
                ps_sx = psst.tile([1, TPC], F32, tag="st")
                ps_sq = psst.tile([1, TPC], F32, tag="st")
                for j in range(HC):
                    s = get_src(j)
                    sqt = lnp.tile([128, TPC], F32, tag="sqt")
                    nc.vector.tensor_mul(sqt[:], s, s)
                    nc.tensor.matmul(ps_sx[:], ones_col[:], s,
                                     start=(j == 0), stop=(j == HC - 1))
                    nc.tensor.matmul(ps_sq[:], ones_col[:], sqt[:],
                                     start=(j == 0), stop=(j == HC - 1))
                mu = lnp.tile([1, TPC], F32, tag="mu")
                m2 = lnp.tile([1, TPC], F32, tag="m2")
                var = lnp.tile([1, TPC], F32, tag="var")
                lnv = lnp.tile([1, TPC], F32, tag="lnv")
                rstd = lnp.tile([1, TPC], F32, tag="rstd")
                mrs = lnp.tile([1, TPC], F32, tag="mrs")
                nc.vector.tensor_scalar_mul(mu[:], ps_sx[:], 1.0 / HID)
                nc.vector.tensor_scalar_mul(m2[:], ps_sq[:], 1.0 / HID)
                nc.vector.tensor_mul(var[:], mu[:], mu[:])
                nc.vector.tensor_sub(var[:], m2[:], var[:])
                nc.scalar.activation(lnv[:], var[:], AF.Ln, bias=eps_b[:])
                nc.scalar.activation(rstd[:], lnv[:], AF.Exp, bias=zero1_b[:],
                                     scale=-0.5)
                nc.vector.tensor_mul(mrs[:], mu[:], rstd[:])
                nc.vector.tensor_scalar_mul(mrs[:], mrs[:], -1.0)
                ps_c1 = psbc.tile([128, TPC], F32, tag="bc")
                ps_c0 = psbc.tile([128, TPC], F32, tag="bc")
                nc.tensor.matmul(ps_c1[:], ones_row[:], rstd[:], start=True, stop=True)
                nc.tensor.matmul(ps_c0[:], ones_row[:], mrs[:], start=True, stop=True)
                for j in range(HC):
                    s = get_src(j)
                    t1 = lnp.tile([128, TPC], F32, tag="t1")
                    t2 = lnp.tile([128, TPC], F32, tag="t2")
                    nc.vector.tensor_mul(t1[:], s, ps_c1[:])
                    nc.vector.tensor_add(t2[:], t1[:], ps_c0[:])
                    put_dst(j, t2, gg[:, j:j + 1], bb[:, j:j + 1])

            # ---------------- Stage A: LN1 (x streamed) + AllGather ----------
            with (
                tc.tile_pool(name="lnA", bufs=3) as lnA,
                tc.tile_pool(name="psstA", bufs=2, space="PSUM") as psstA,
                tc.tile_pool(name="psbcA", bufs=2, space="PSUM") as psbcA,
            ):
                def get_x(j):
                    t = lnA.tile([128, TPC], F32, tag="xs")
                    nc.sync.dma_start(t[:], xT[j * 128:(j + 1) * 128, :])
                    return t[:]

                def put_h1(j, t2, gj, bj):
                    hc_t = lnA.tile([128, TPC], F16, tag="hc")
                    nc.gpsimd.tensor_scalar(hc_t[:], t2[:], gj, bj, MULT, ADD)
                    tgt = ag_in_a if j < 8 else ag_in_b
                    jj = j % 8
                    nc.sync.dma_start(tgt[jj * 128:(jj + 1) * 128, :], hc_t[:])

                layernorm(get_x, put_h1, g1_sb, b1_sb, lnA, psstA, psbcA)

            H2 = HID // 2
            if single_core:
                # timing stand-in for AllGather (~real collective cost): one
                # quarter-width write per rank slot establishes deps + ~15us
                for r in range(NCORES):
                    nc.sync.dma_start(ag_out_a[r * H2:(r + 1) * H2, 0:TPC // 4],
                                      ag_in_a[:, 0:TPC // 4])
                    nc.sync.dma_start(ag_out_b[r * H2:(r + 1) * H2, 0:TPC // 4],
                                      ag_in_b[:, 0:TPC // 4])
            else:
                nc.gpsimd.collective_compute(
                    "AllGather", mybir.AluOpType.bypass, replica_groups=rg,
                    ins=[ag_in_a.opt()], outs=[ag_out_a],
                )
                nc.gpsimd.collective_compute(
                    "AllGather", mybir.AluOpType.bypass, replica_groups=rg,
                    ins=[ag_in_b.opt()], outs=[ag_out_b],
                )

            with tc.tile_pool(name="qkv", bufs=1) as qkv:
                qr_sb = qkv.tile([128, NH * TOK], F16, tag="qr")
                kr_sb = qkv.tile([128, NH * TOK], F16, tag="kr")
                v_sb = qkv.tile([128, (TOK // 128) * NH * 128], F16, tag="v")
                rC = qkv.tile([128, TOK], F16, tag="rC")
                rS = qkv.tile([128, TOK], F16, tag="rS")
                nc.sync.dma_start(rC[:], ropeC[:])
                nc.sync.dma_start(rS[:], ropeS[:])
                wq_sb = qkv.tile([128, HC * NH * 128], F16, tag="wq")
                wk_sb = qkv.tile([128, HC * NH * 128], F16, tag="wk")
                wv_sb = qkv.tile([128, HC * NH * 128], F16, tag="wv")
                nc.scalar.dma_start(wq_sb[:], wq[:])
                nc.scalar.dma_start(wk_sb[:], wk[:])
                nc.scalar.dma_start(wv_sb[:], wv[:])

                # ---------------- Stage B: qkv projections + RoPE ------------
                with (
                    tc.tile_pool(name="htc", bufs=30) as htc,
                    tc.tile_pool(name="qkpre", bufs=6) as qkpre,
                    tc.tile_pool(name="ropet", bufs=8) as ropet,
                    tc.tile_pool(name="psqk", bufs=4, space="PSUM") as psqk,
                    tc.tile_pool(name="psv", bufs=4, space="PSUM") as psv,
                ):
                    for tb in range(NCORES):
                        hts = []
                        for j in range(HC):
                            t = htc.tile([128, TPC], F16, tag="ht")
                            buf = ag_out_a if j < 8 else ag_out_b
                            jj = j % 8
                            nc.sync.dma_start(
                                t[:],
                                buf[tb * (HID // 2) + jj * 128:
                                    tb * (HID // 2) + (jj + 1) * 128, :],
                            )
                            hts.append(t)
                        for (w_sb, r_sb) in ((wq_sb, qr_sb), (wk_sb, kr_sb)):
                            for m in range(NH):
                                ps = psqk.tile([128, TPC], F32, tag="qk")
                                for j in range(HC):
                                    nc.tensor.matmul(
                                        ps[:],
                                        w_sb[:, j * (NH * 128) + m * 128:
                                             j * (NH * 128) + (m + 1) * 128],
                                        hts[j][:],
                                        start=(j == 0), stop=(j == HC - 1),
                                    )
                                pre = qkpre.tile([128, TPC], F16, tag="pre")
                                nc.scalar.activation(pre[:], ps[:], AF.Copy)
                                # RoPE: rows [0:64] even dims, [64:128] odd dims
                                col = m * TOK + tb * TPC
                                cs = slice(tb * TPC, (tb + 1) * TPC)
                                qe = pre[0:64, :]
                                qo = pre[64:128, :]
                                t1 = ropet.tile([64, TPC], F16, tag="t1")
                                t2 = ropet.tile([64, TPC], F16, tag="t2")
                                t3 = ropet.tile([64, TPC], F16, tag="t3")
                                t4 = ropet.tile([64, TPC], F16, tag="t4")
                                nc.vector.tensor_mul(t1[:], qe, rC[0:64, cs])
                                nc.vector.tensor_mul(t2[:], qo, rS[64:128, cs])
                                nc.vector.tensor_sub(
                                    r_sb[0:64, col:col + TPC], t1[:], t2[:])
                                nc.vector.tensor_mul(t3[:], qe, rS[0:64, cs])
                                nc.vector.tensor_mul(t4[:], qo, rC[64:128, cs])
                                nc.vector.tensor_add(
                                    r_sb[64:128, col:col + TPC], t3[:], t4[:])
                        for mt in range(4):
                            ps = psv.tile([128, NH * 128], F32, tag="v")
                            for j in range(HC):
                                nc.tensor.matmul(
                                    ps[:],
                                    hts[j][:, mt * 128:(mt + 1) * 128],
                                    wv_sb[:, j * (NH * 128):(j + 1) * (NH * 128)],
                                    start=(j == 0), stop=(j == HC - 1),
                                )
                            ti = tb * 4 + mt
                            nc.scalar.activation(
                                v_sb[:, ti * (NH * 128):(ti + 1) * (NH * 128)],
                                ps[:], AF.Copy)

                # ---------------- Stage C: attention -------------------------
                SB = S // TPC   # 4 query blocks per batch
                KCN = S // 128  # 16 key chunks per batch
                with (
                    tc.tile_pool(name="cp", bufs=5) as cp,
                    tc.tile_pool(name="pss", bufs=2, space="PSUM") as pss_p,
                    tc.tile_pool(name="pso", bufs=3, space="PSUM") as pso_p,
                    tc.tile_pool(name="psdn", bufs=1, space="PSUM") as psdn_p,
                ):
                    for m in range(NH):
                      for b in range(B):
                        if True:
                            qcol = m * TOK + b * S
                            for qb in range(SB):
                                pso = pso_p.tile([128, TPC], F32, tag="o")
                                den = cp.tile([128, TPC], F16, tag="den")
                                qsl = slice(qcol + qb * TPC, qcol + (qb + 1) * TPC)
                                for kg in range(KCN // 2):
                                    pss = pss_p.tile([128, 2 * TPC], F32, tag="s")
                                    for h_ in range(2):
                                        kc = kg * 2 + h_
                                        nc.tensor.matmul(
                                            pss[:, h_ * TPC:(h_ + 1) * TPC],
                                            kr_sb[:, qcol + kc * 128: qcol + (kc + 1) * 128],
                                            qr_sb[:, qsl],
                                            start=True, stop=True,
                                        )
                                    pt = cp.tile([128, 2 * TPC], F16, tag="pt")
                                    nc.scalar.activation(
                                        pt[:], pss[:], AF.Exp, scale=SCALE,
                                        bias=expb_b[:])
                                    if kg == 0:
                                        nc.vector.tensor_add(
                                            den[:], pt[:, 0:TPC], pt[:, TPC:2 * TPC])
                                    else:
                                        nc.vector.tensor_add(den[:], den[:], pt[:, 0:TPC])
                                        nc.vector.tensor_add(den[:], den[:], pt[:, TPC:2 * TPC])
                                    for h_ in range(2):
                                        kc = kg * 2 + h_
                                        ti = b * (S // 128) + kc
                                        nc.tensor.matmul(
                                            pso[:],
                                            v_sb[:, ti * (NH * 128) + m * 128:
                                                 ti * (NH * 128) + (m + 1) * 128],
                                            pt[:, h_ * TPC:(h_ + 1) * TPC],
                                            start=(kc == 0), stop=(kc == KCN - 1),
                                        )
                                psden = psdn_p.tile([1, TPC], F32, tag="dn")
                                nc.tensor.matmul(psden[:], ones_col16[:], den[:],
                                                 start=True, stop=True)
                                rec = cp.tile([1, TPC], F32, tag="rec")
                                nc.vector.reciprocal(rec[:], psden[:])
                                rb = cp.tile([128, TPC], F32, tag="rbs")
                                nc.gpsimd.partition_broadcast(rb[:], rec[:])
                                at = cp.tile([128, TPC], F16, tag="at")
                                nc.vector.tensor_mul(at[:], pso[:], rb[:])
                                row = (b * SB + qb) * 128
                                nc.sync.dma_start(
                                    a2a_in_m[m][row:row + 128, :], at[:])
                        if b == B - 1:
                            if single_core:
                                a2a_mid = dram.tile([NCORES * 128, TPC], F16,
                                                    name=f"a2am{m}")
                                nc.sync.dma_start(a2a_mid[:, :], a2a_in_m[m][:, :])
                                nc.sync.dma_start(a2a_out_m[m][:, :], a2a_mid[:, :])
                            else:
                                nc.gpsimd.collective_compute(
                                    "AllToAll", mybir.AluOpType.bypass,
                                    replica_groups=rg,
                                    ins=[a2a_in_m[m].opt()], outs=[a2a_out_m[m].opt()],
                                )

            with tc.tile_pool(name="late", bufs=1) as late:
                x2_sb = late.tile([128, HC * TPC], F32, tag="x2")
                h2_sb = late.tile([128, HC * TPC], F16, tag="h2")
                ff_sb = late.tile([128, FFC * TPC], F16, tag="ff")

                # ------------- Stage D: out_proj + residual + LN2 ------------
                with (
                    tc.tile_pool(name="atp", bufs=HC + 2) as atp,
                    tc.tile_pool(name="wop", bufs=4) as wop,
                    tc.tile_pool(name="lnD", bufs=4) as lnD,
                    tc.tile_pool(name="pso2", bufs=4, space="PSUM") as pso2_p,
                    tc.tile_pool(name="psstD", bufs=2, space="PSUM") as psstD,
                    tc.tile_pool(name="psbcD", bufs=2, space="PSUM") as psbcD,
                ):
                    ats = []
                    for j in range(HC):
                        t = atp.tile([128, TPC], F16, tag="at")
                        buf = a2a_out_m[j % 2]
                        r = j // 2
                        nc.sync.dma_start(t[:], buf[r * 128:(r + 1) * 128, :])
                        ats.append(t)
                    for mo in range(HC):
                        ws = wop.tile([128, HC * 128], F16, tag="wo")
                        nc.scalar.dma_start(ws[:], wo[mo * 128:(mo + 1) * 128, :])
                        ps = pso2_p.tile([128, TPC], F32, tag="o2")
                        for j in range(HC):
                            nc.tensor.matmul(
                                ps[:], ws[:, j * 128:(j + 1) * 128], ats[j][:],
                                start=(j == 0), stop=(j == HC - 1),
                            )
                        xt = lnD.tile([128, TPC], F32, tag="xres")
                        nc.sync.dma_start(xt[:], xT[mo * 128:(mo + 1) * 128, :])
                        nc.vector.tensor_add(
                            x2_sb[:, mo * TPC:(mo + 1) * TPC], ps[:], xt[:])

                    def get_x2(j):
                        return x2_sb[:, j * TPC:(j + 1) * TPC]

                    def put_h2(j, t2, gj, bj):
                        nc.gpsimd.tensor_scalar(
                            h2_sb[:, j * TPC:(j + 1) * TPC], t2[:], gj, bj,
                            MULT, ADD)

                    layernorm(get_x2, put_h2, g2_sb, b2_sb, lnD, psstD, psbcD)

                # ------------- Stage E: MLP ----------------------------------
                with (
                    tc.tile_pool(name="wf1p", bufs=4) as wf1p,
                    tc.tile_pool(name="wf2p", bufs=5) as wf2p,
                    tc.tile_pool(name="outp", bufs=3) as outp,
                    tc.tile_pool(name="psf1", bufs=4, space="PSUM") as psf1_p,
                    tc.tile_pool(name="psf2", bufs=4, space="PSUM") as psf2_p,
                ):
                    for mo in range(FFC):
                        ws = wf1p.tile([128, HC * 128], F16, tag="wf1")
                        nc.scalar.dma_start(ws[:], wf1[mo * 128:(mo + 1) * 128, :])
                        ps = psf1_p.tile([128, TPC], F32, tag="f1")
                        for j in range(HC):
                            nc.tensor.matmul(
                                ps[:], ws[:, j * 128:(j + 1) * 128],
                                h2_sb[:, j * TPC:(j + 1) * TPC],
                                start=(j == 0), stop=(j == HC - 1),
                            )
                        nc.scalar.activation(
                            ff_sb[:, mo * TPC:(mo + 1) * TPC], ps[:], AF.Gelu,
                            bias=zero_b[:])
                    for mo in range(HC):
                        ws = wf2p.tile([128, FFC * 128], F16, tag="wf2")
                        nc.scalar.dma_start(ws[:], wf2[mo * 128:(mo + 1) * 128, :])
                        ps = psf2_p.tile([128, TPC], F32, tag="f2")
                        for j in range(FFC):
                            nc.tensor.matmul(
                                ps[:], ws[:, j * 128:(j + 1) * 128],
                                ff_sb[:, j * TPC:(j + 1) * TPC],
                                start=(j == 0), stop=(j == FFC - 1),
                            )
                        ot = outp.tile([128, TPC], F32, tag="ot")
                        nc.vector.tensor_add(
                            ot[:], ps[:], x2_sb[:, mo * TPC:(mo + 1) * TPC])
                        nc.sync.dma_start(outT[mo * 128:(mo + 1) * 128, :], ot[:])
    return nc


def _build():
    if "nc" in _CACHE:
        return _CACHE["nc"]
    nc = bacc.Bacc(
        "TRN2", target_bir_lowering=False, debug=False,
        enable_asserts=True, num_devices=NCORES,
    )
    _emit(nc)
    nc.compile()
    _CACHE["nc"] = nc
    return nc


def _strips(wT, n_strips):
    # wT [K, n_strips*128] -> [n_strips*128, K] where strip m rows are
    # [128 partitions, K/128 chunks * 128] in SBUF lhsT layout
    K = wT.shape[0]
    kc = K // 128
    out = np.empty((n_strips * 128, K), dtype=np.float16)
    for m_ in range(n_strips):
        s = wT[:, m_ * 128:(m_ + 1) * 128]          # [K, 128]
        s = s.reshape(kc, 128, 128).transpose(1, 0, 2).reshape(128, K)
        out[m_ * 128:(m_ + 1) * 128, :] = s
    return out


def prepare_inputs(x, pe, w_qkv, w_out, w_fc1, w_fc2, g1, b1, g2, b2):
    x = np.asarray(x, np.float32)
    pe = np.asarray(pe, np.float32)
    w_qkv = np.asarray(w_qkv, np.float32)
    w_out = np.asarray(w_out, np.float32)
    w_fc1 = np.asarray(w_fc1, np.float32)
    w_fc2 = np.asarray(w_fc2, np.float32)

    xf = x.reshape(TOK, HID)
    perm = np.r_[np.arange(0, 128, 2), np.arange(1, 128, 2)]

    ropeC = np.tile(pe[:, 0::2].T, (2, B)).astype(np.float16)   # [128, TOK]
    ropeS = np.tile(pe[:, 1::2].T, (2, B)).astype(np.float16)

    gb = [np.asarray(v, np.float32).reshape(HC, 128).T.copy()
          for v in (g1, b1, g2, b2)]

    wo_h = _strips(w_out.T.astype(np.float16), HC)        # w_out.T: [feat, out]
    wf1_h = _strips(w_fc1.T.astype(np.float16), FFC)      # [hid, ffn]
    wf2_h = _strips(w_fc2.T.astype(np.float16), HC)       # [ffn, hid]

    in_maps = []
    for c in range(NCORES):
        heads = [NH * c + i for i in range(NH)]
        # q/k rows with per-head even/odd permutation; v natural
        qrows = np.concatenate([w_qkv[h * D + perm] for h in heads])      # [256, HID]
        krows = np.concatenate([w_qkv[HID + h * D + perm] for h in heads])
        vrows = np.concatenate([w_qkv[2 * HID + h * D: 2 * HID + (h + 1) * D]
                                for h in heads])

        def wlay(rows):
            # rows [NH*128, HID] -> lhsT sbuf layout [128, HC, NH*128]
            t = rows.T.astype(np.float16)                  # [HID, NH*128]
            t = t.reshape(HC, 128, NH * 128).transpose(1, 0, 2)
            return t.reshape(128, HC * NH * 128)

        xTc = np.ascontiguousarray(xf[c * TPC:(c + 1) * TPC].T)  # [HID, TPC]
        in_maps.append({
            "xT": xTc,
            "wq": wlay(qrows), "wk": wlay(krows), "wv": wlay(vrows),
            "wo": wo_h, "wf1": wf1_h, "wf2": wf2_h,
            "g1": gb[0], "b1": gb[1], "g2": gb[2], "b2": gb[3],
            "ropeC": ropeC, "ropeS": ropeS,
        })
    return in_maps


def run(in_maps, **kwargs):
    nc = _build()
    return bass_utils.run_bass_kernel_spmd(
        nc, in_maps, core_ids=list(range(NCORES)), **kwargs
    )


def kernel(x, pe, w_qkv, w_out, w_fc1, w_fc2, g1, b1, g2, b2):
    in_maps = prepare_inputs(x, pe, w_qkv, w_out, w_fc1, w_fc2, g1, b1, g2, b2)
    res = run(in_maps)
    fullT = np.concatenate([res.results[c]["outT"] for c in range(NCORES)], axis=1)
    return np.ascontiguousarray(fullT.T).reshape(B, S, HID).astype(np.float32)



# revision 6
# speedup vs baseline: 1.3366x; 1.3366x over previous
"""MiniTransformerLayer on 8 Trainium2 NeuronCores — fp8 DoubleRow version.

Sharding (identical collective structure to the fp16 baseline):
  - tokens t = b*S + s flattened to [4096]; core c owns tokens [512c, 512(c+1))
    and heads {2c, 2c+1}; LN1 on own shard -> AllGather h (fp8),
    head-sharded attention, AllToAll attn output (fp8), then data-parallel
    out_proj/MLP on own 512 tokens.

Precision plan (validated vs the jax reference in numpy, rel_err ~4e-3):
  - All big matmuls run fp8e4m3 with DoubleRow perf mode (2 contraction
    planes of 128 per instruction, 0.5 cycles/row).
  - Weights host-prescaled x16 before fp8 quantization (keeps them out of
    the subnormal range); the 1/16 folds into the psum-evacuation copies.
  - qkv / attn@V / out_proj: single-level fp8 operands (attention output is
    ~20x smaller than the residual, so its quantization error is diluted).
  - scores: k is two-level (k_hi + k_lo ride the two DoubleRow planes, with
    q duplicated on the moving side) — exact-k at no extra PE cost.
  - fc1/fc2: three-pass two-level (w_hi@x_hi + w_lo@x_hi + w_hi@x_lo) —
    the MLP dominates the output (sigma ~0.86 of 1.32) and needs it.
  - attn values prescaled x8 (folded into the 1/den reciprocal) before fp8.
  - LayerNorm stats via fp16 ones-matmuls (1 cyc/row instead of fp32's 4).
"""

import sys

sys.path.insert(0, "/opt/trn_rl_repo")

import numpy as np

import concourse.bass as bass
import concourse.bacc as bacc
import concourse.tile as tile
import concourse.mybir as mybir
from concourse import bass_utils

F8 = mybir.dt.float8e4
F16 = mybir.dt.float16
F32 = mybir.dt.float32
AF = mybir.ActivationFunctionType
DR = mybir.MatmulPerfMode.DoubleRow

NCORES = 8
B, S, HID, HEADS, D, FFN = 2, 2048, 2048, 16, 128, 4096
TOK = B * S            # 4096 flat tokens
TPC = TOK // NCORES    # 512 tokens per core
HC = HID // 128        # 16 hidden chunks
PAIRS = HC // 2        # 8 hidden chunk-pairs
FFC = FFN // 128       # 32 ffn chunks
FPAIRS = FFC // 2      # 16 ffn chunk-pairs
NH = HEADS // NCORES   # 2 heads per core
SCALE = 1.0 / float(np.sqrt(D))
EXP_BIAS = -3.0
EPS = 1e-5
WS = 16.0              # weight prescale (host) folded out in psum copies
AS = 8.0               # attn prescale folded into 1/den

_CACHE = {}


def _pair2(ap):
    """[128, 2N] AP -> [128, 2, N] DoubleRow view."""
    return ap.rearrange("k (two n) -> k two n", two=2)


def _emit(nc, single_core=False):
    xT = nc.dram_tensor("xT", [HID, TPC], F32, kind="ExternalInput")
    wq = nc.dram_tensor("wq", [128, PAIRS * NH * 2 * 128], F8, kind="ExternalInput")
    wk = nc.dram_tensor("wk", [128, PAIRS * NH * 2 * 128], F8, kind="ExternalInput")
    wv = nc.dram_tensor("wv", [128, PAIRS * 2 * NH * 128], F8, kind="ExternalInput")
    wo = nc.dram_tensor("wo", [HC * 128, HID], F8, kind="ExternalInput")
    wf1 = nc.dram_tensor("wf1", [FFC * 128, 2 * HID], F8, kind="ExternalInput")
    wf2 = nc.dram_tensor("wf2", [HC * 128, 2 * FFN], F8, kind="ExternalInput")
    g1 = nc.dram_tensor("g1", [128, HC], F32, kind="ExternalInput")
    b1 = nc.dram_tensor("b1", [128, HC], F32, kind="ExternalInput")
    g2 = nc.dram_tensor("g2", [128, HC], F32, kind="ExternalInput")
    b2 = nc.dram_tensor("b2", [128, HC], F32, kind="ExternalInput")
    ropeC = nc.dram_tensor("ropeC", [128, TOK], F16, kind="ExternalInput")
    ropeS = nc.dram_tensor("ropeS", [128, TOK], F16, kind="ExternalInput")
    outT = nc.dram_tensor("outT", [HID, TPC], F32, kind="ExternalOutput")

    rg = [list(range(NCORES))]
    MULT, ADD = mybir.AluOpType.mult, mybir.AluOpType.add
    H2 = HID // 2

    with tile.TileContext(nc) as tc:
        with (
            tc.tile_pool(name="const", bufs=1) as const,
            tc.tile_pool(name="dram", bufs=1, space="DRAM") as dram,
        ):
            ones_c16 = const.tile([128, 1], F16, tag="onc16")
            nc.vector.memset(ones_c16[:], 1.0)
            ones_r16 = const.tile([1, 128], F16, tag="onr16")
            nc.vector.memset(ones_r16[:], 1.0)
            # DoubleRow weight APs need plane stride %16 bytes: pad to [128,2,16]
            ones8dr = const.tile([128, 32], F8, tag="on8")
            nc.vector.memset(ones8dr[:], 1.0)
            eps_b = const.tile([1, 1], F32, tag="epsb")
            nc.vector.memset(eps_b[:], EPS)
            zero1_b = const.tile([1, 1], F32, tag="z1b")
            nc.vector.memset(zero1_b[:], 0.0)
            zero_b = const.tile([128, 1], F32, tag="zb")
            nc.vector.memset(zero_b[:], 0.0)
            expb_b = const.tile([128, 1], F32, tag="expb")
            nc.vector.memset(expb_b[:], EXP_BIAS)
            g1_sb = const.tile([128, HC], F32, tag="g1")
            b1_sb = const.tile([128, HC], F32, tag="b1")
            g2_sb = const.tile([128, HC], F32, tag="g2")
            b2_sb = const.tile([128, HC], F32, tag="b2")
            nc.scalar.dma_start(g1_sb[:], g1[:])
            nc.scalar.dma_start(b1_sb[:], b1[:])
            nc.scalar.dma_start(g2_sb[:], g2[:])
            nc.scalar.dma_start(b2_sb[:], b2[:])

            ag_in_a = dram.tile([H2, TPC], F8)
            ag_in_b = dram.tile([H2, TPC], F8)
            a2a_in_m = [dram.tile([NCORES * 128, TPC], F8, name=f"a2ai{m}")
                        for m in range(NH)]
            a2a_out_m = [dram.tile([NCORES * 128, TPC], F8,
                                   name=f"a2ao{m}") for m in range(NH)]
            if single_core:
                ag_out_a = dram.tile([NCORES * H2, TPC], F8)
                ag_out_b = dram.tile([NCORES * H2, TPC], F8)
            else:
                ag_out_a = nc.dram_tensor(
                    "ag_out_a_sh", [NCORES * H2, TPC], F8,
                    addr_space="Shared").ap()
                ag_out_b = nc.dram_tensor(
                    "ag_out_b_sh", [NCORES * H2, TPC], F8,
                    addr_space="Shared").ap()

            # ---------------- Stage A: LN1 (x streamed) + AllGather ----------
            with (
                tc.tile_pool(name="lnx", bufs=HC) as lnx,
                tc.tile_pool(name="lnA", bufs=3) as lnA,
                tc.tile_pool(name="psstA", bufs=2, space="PSUM") as psstA,
                tc.tile_pool(name="psbcA", bufs=2, space="PSUM") as psbcA,
            ):
                ps_sx = psstA.tile([1, TPC], F32, tag="st")
                ps_sq = psstA.tile([1, TPC], F32, tag="st")
                x16s = []
                for j in range(HC):
                    xj = lnA.tile([128, TPC], F32, tag="xs")
                    nc.sync.dma_start(xj[:], xT[j * 128:(j + 1) * 128, :])
                    x16 = lnx.tile([128, TPC], F16, tag="x16")
                    nc.vector.tensor_copy(x16[:], xj[:])
                    sq = lnA.tile([128, TPC], F16, tag="sq")
                    nc.vector.tensor_mul(sq[:], x16[:], x16[:])
                    nc.tensor.matmul(ps_sx[:], ones_c16[:], x16[:],
                                     start=(j == 0), stop=(j == HC - 1))
                    nc.tensor.matmul(ps_sq[:], ones_c16[:], sq[:],
                                     start=(j == 0), stop=(j == HC - 1))
                    x16s.append(x16)
                mu = lnA.tile([1, TPC], F32, tag="mu")
                m2 = lnA.tile([1, TPC], F32, tag="m2")
                var = lnA.tile([1, TPC], F32, tag="var")
                lnv = lnA.tile([1, TPC], F32, tag="lnv")
                rstd = lnA.tile([1, TPC], F16, tag="rstd")
                mrs = lnA.tile([1, TPC], F32, tag="mrs")
                mrs16 = lnA.tile([1, TPC], F16, tag="mrs16")
                nc.vector.tensor_scalar_mul(mu[:], ps_sx[:], 1.0 / HID)
                nc.vector.tensor_scalar_mul(m2[:], ps_sq[:], 1.0 / HID)
                nc.vector.tensor_mul(var[:], mu[:], mu[:])
                nc.vector.tensor_sub(var[:], m2[:], var[:])
                nc.scalar.activation(lnv[:], var[:], AF.Ln, bias=eps_b[:])
                nc.scalar.activation(rstd[:], lnv[:], AF.Exp, bias=zero1_b[:],
                                     scale=-0.5)
                nc.vector.tensor_mul(mrs[:], mu[:], rstd[:])
                nc.vector.tensor_scalar_mul(mrs16[:], mrs[:], -1.0)
                ps_c1 = psbcA.tile([128, TPC], F32, tag="bc")
                ps_c0 = psbcA.tile([128, TPC], F32, tag="bc")
                nc.tensor.matmul(ps_c1[:], ones_r16[:], rstd[:], start=True,
                                 stop=True)
                nc.tensor.matmul(ps_c0[:], ones_r16[:], mrs16[:], start=True,
                                 stop=True)
                for j in range(HC):
                    t1 = lnA.tile([128, TPC], F32, tag="t1")
                    t2 = lnA.tile([128, TPC], F32, tag="t2")
                    nc.vector.tensor_mul(t1[:], x16s[j][:], ps_c1[:])
                    nc.vector.tensor_add(t2[:], t1[:], ps_c0[:])
                    h8 = lnA.tile([128, TPC], F8, tag="h8")
                    nc.gpsimd.tensor_scalar(h8[:], t2[:], g1_sb[:, j:j + 1],
                                            b1_sb[:, j:j + 1], MULT, ADD)
                    tgt = ag_in_a if j < 8 else ag_in_b
                    jj = j % 8
                    nc.sync.dma_start(tgt[jj * 128:(jj + 1) * 128, :], h8[:])

            if single_core:
                # timing stand-in for AllGather (~real collective cost)
                for r in range(NCORES):
                    nc.sync.dma_start(ag_out_a[r * H2:(r + 1) * H2, 0:TPC // 4],
                                      ag_in_a[:, 0:TPC // 4])
                    nc.sync.dma_start(ag_out_b[r * H2:(r + 1) * H2, 0:TPC // 4],
                                      ag_in_b[:, 0:TPC // 4])
            else:
                nc.gpsimd.collective_compute(
                    "AllGather", mybir.AluOpType.bypass, replica_groups=rg,
                    ins=[ag_in_a.opt()], outs=[ag_out_a],
                )
                nc.gpsimd.collective_compute(
                    "AllGather", mybir.AluOpType.bypass, replica_groups=rg,
                    ins=[ag_in_b.opt()], outs=[ag_out_b],
                )

            with tc.tile_pool(name="qkv", bufs=1) as qkv:
                # qr2: q duplicated in both DoubleRow planes (cols 0/8192)
                # kr2: plane0 = k_hi, plane1 = k_lo (two-level k)
                qr2 = qkv.tile([128, 2 * NH * TOK], F8, tag="qr")
                kr2 = qkv.tile([128, 2 * NH * TOK], F8, tag="kr")
                v2 = qkv.tile([128, (TOK // 128) * NH * 128], F8, tag="v")
                rC = qkv.tile([128, TOK], F16, tag="rC")
                rS = qkv.tile([128, TOK], F16, tag="rS")
                nc.sync.dma_start(rC[:], ropeC[:])
                nc.sync.dma_start(rS[:], ropeS[:])
                wq_sb = qkv.tile([128, PAIRS * NH * 2 * 128], F8, tag="wq")
                wk_sb = qkv.tile([128, PAIRS * NH * 2 * 128], F8, tag="wk")
                wv_sb = qkv.tile([128, PAIRS * 2 * NH * 128], F8, tag="wv")
                nc.scalar.dma_start(wq_sb[:], wq[:])
                nc.scalar.dma_start(wk_sb[:], wk[:])
                nc.scalar.dma_start(wv_sb[:], wv[:])

                # ---------------- Stage B: qkv projections + RoPE ------------
                with (
                    tc.tile_pool(name="htp", bufs=2) as htp,
                    tc.tile_pool(name="qkpre", bufs=4) as qkpre,
                    tc.tile_pool(name="ropet", bufs=8) as ropet,
                    tc.tile_pool(name="psqk", bufs=4, space="PSUM") as psqk,
                    tc.tile_pool(name="psv", bufs=4, space="PSUM") as psv,
                ):
                    for tb in range(NCORES):
                        h_t = htp.tile([128, PAIRS * 1024], F8, tag="ht")
                        for ch in range(HC):
                            buf = ag_out_a if ch < 8 else ag_out_b
                            r0 = tb * H2 + (ch % 8) * 128
                            nc.sync.dma_start(
                                h_t[:, ch * TPC:(ch + 1) * TPC],
                                buf[r0:r0 + 128, :],
                            )
                        for (w_sb, is_q) in ((wq_sb, True), (wk_sb, False)):
                            for m in range(NH):
                                ps = psqk.tile([128, TPC], F32, tag="qk")
                                for p in range(PAIRS):
                                    c0 = (p * NH + m) * 256
                                    nc.tensor.matmul(
                                        ps[:],
                                        _pair2(w_sb[:, c0:c0 + 256]),
                                        _pair2(h_t[:, p * 1024:(p + 1) * 1024]),
                                        start=(p == 0), stop=(p == PAIRS - 1),
                                        perf_mode=DR,
                                    )
                                pre = qkpre.tile([128, TPC], F16, tag="pre")
                                nc.scalar.activation(pre[:], ps[:], AF.Copy,
                                                     scale=1.0 / WS)
                                col = (m * 8 + tb) * TPC
                                cs = slice(tb * TPC, (tb + 1) * TPC)
                                qe = pre[0:64, :]
                                qo = pre[64:128, :]
                                t1 = ropet.tile([64, TPC], F16, tag="t1")
                                t2 = ropet.tile([64, TPC], F16, tag="t2")
                                t3 = ropet.tile([64, TPC], F16, tag="t3")
                                t4 = ropet.tile([64, TPC], F16, tag="t4")
                                nc.vector.tensor_mul(t1[:], qe, rC[0:64, cs])
                                nc.vector.tensor_mul(t2[:], qo, rS[64:128, cs])
                                nc.vector.tensor_mul(t3[:], qe, rS[0:64, cs])
                                nc.vector.tensor_mul(t4[:], qo, rC[64:128, cs])
                                if is_q:
                                    nc.vector.tensor_sub(
                                        qr2[0:64, col:col + TPC], t1[:], t2[:])
                                    nc.vector.tensor_add(
                                        qr2[64:128, col:col + TPC], t3[:], t4[:])
                                    nc.vector.tensor_copy(
                                        qr2[:, NH * TOK + col:
                                            NH * TOK + col + TPC],
                                        qr2[:, col:col + TPC])
                                else:
                                    kt = ropet.tile([128, TPC], F16, tag="kt")
                                    nc.vector.tensor_sub(
                                        kt[0:64, :], t1[:], t2[:])
                                    nc.vector.tensor_add(
                                        kt[64:128, :], t3[:], t4[:])
                                    nc.vector.tensor_copy(
                                        kr2[:, col:col + TPC], kt[:])
                                    nc.vector.tensor_sub(
                                        kr2[:, NH * TOK + col:
                                            NH * TOK + col + TPC],
                                        kt[:], kr2[:, col:col + TPC])
                        for mt in range(4):
                            psvt = psv.tile([128, NH * 128], F32, tag="v")
                            for p in range(PAIRS):
                                nc.tensor.matmul(
                                    psvt[:],
                                    _pair2(h_t[:, p * 1024:(p + 1) * 1024])
                                    [:, :, mt * 128:(mt + 1) * 128],
                                    _pair2(wv_sb[:, p * 512:(p + 1) * 512]),
                                    start=(p == 0), stop=(p == PAIRS - 1),
                                    perf_mode=DR,
                                )
                            ti = tb * 4 + mt
                            nc.scalar.activation(
                                v2[:, ti * (NH * 128):(ti + 1) * (NH * 128)],
                                psvt[:], AF.Copy, scale=1.0 / WS)

                # ---------------- Stage C: attention -------------------------
                SB = S // TPC   # 4 query blocks per batch
                KCN = S // 128  # 16 key chunks per batch
                qr_dr = _pair2(qr2[:])
                kr_dr = _pair2(kr2[:])
                with (
                    tc.tile_pool(name="cp", bufs=5) as cp,
                    tc.tile_pool(name="pss", bufs=2, space="PSUM") as pss_p,
                    tc.tile_pool(name="pso", bufs=3, space="PSUM") as pso_p,
                    tc.tile_pool(name="psdn", bufs=1, space="PSUM") as psdn_p,
                ):
                    for m in range(NH):
                        for b in range(B):
                            qcol0 = (m * 8 + b * 4) * TPC
                            for qb in range(SB):
                                pso = pso_p.tile([128, TPC], F32, tag="o")
                                psden = psdn_p.tile([1, TPC], F32, tag="dn")
                                qsl = qr_dr[:, :, qcol0 + qb * TPC:
                                            qcol0 + (qb + 1) * TPC]
                                for kg in range(KCN // 2):
                                    pss = pss_p.tile([128, 2 * TPC], F32,
                                                     tag="s")
                                    for h_ in range(2):
                                        kc = kg * 2 + h_
                                        kcol = qcol0 + (kc // 4) * TPC \
                                            + (kc % 4) * 128
                                        nc.tensor.matmul(
                                            pss[:, h_ * TPC:(h_ + 1) * TPC],
                                            kr_dr[:, :, kcol:kcol + 128],
                                            qsl,
                                            start=True, stop=True,
                                            perf_mode=DR,
                                        )
                                    pt = cp.tile([128, 2 * TPC], F8, tag="pt")
                                    nc.scalar.activation(
                                        pt[:], pss[:], AF.Exp, scale=SCALE,
                                        bias=expb_b[:])
                                    ptr = _pair2(pt[:])
                                    nc.tensor.matmul(
                                        psden[:],
                                        _pair2(ones8dr[:])[:, :, 0:1], ptr,
                                        start=(kg == 0),
                                        stop=(kg == KCN // 2 - 1),
                                        perf_mode=DR,
                                    )
                                    ti0 = b * 16 + 2 * kg
                                    vap = _pair2(
                                        v2[:, ti0 * 256:ti0 * 256 + 512]
                                    )[:, :, m * 128:(m + 1) * 128]
                                    nc.tensor.matmul(
                                        pso[:], vap, ptr,
                                        start=(kg == 0),
                                        stop=(kg == KCN // 2 - 1),
                                        perf_mode=DR,
                                    )
                                rec = cp.tile([1, TPC], F32, tag="rec")
                                nc.vector.reciprocal(rec[:], psden[:])
                                rec8 = cp.tile([1, TPC], F32, tag="rec8")
                                nc.vector.tensor_scalar_mul(rec8[:], rec[:], AS)
                                rb = cp.tile([128, TPC], F32, tag="rbs")
                                nc.gpsimd.partition_broadcast(rb[:], rec8[:])
                                at = cp.tile([128, TPC], F8, tag="at")
                                nc.vector.tensor_mul(at[:], pso[:], rb[:])
                                row = (b * SB + qb) * 128
                                nc.sync.dma_start(
                                    a2a_in_m[m][row:row + 128, :], at[:])
                            if b == B - 1:
                                if single_core:
                                    a2a_mid = dram.tile(
                                        [NCORES * 128, TPC], F8,
                                        name=f"a2am{m}")
                                    nc.sync.dma_start(a2a_mid[:, :],
                                                      a2a_in_m[m][:, :])
                                    nc.sync.dma_start(a2a_out_m[m][:, :],
                                                      a2a_mid[:, :])
                                else:
                                    nc.gpsimd.collective_compute(
                                        "AllToAll", mybir.AluOpType.bypass,
                                        replica_groups=rg,
                                        ins=[a2a_in_m[m].opt()],
                                        outs=[a2a_out_m[m].opt()],
                                    )

            with tc.tile_pool(name="late", bufs=1) as late:
                x2_sb = late.tile([128, HC * TPC], F16, tag="x2")
                h2_2 = late.tile([128, 2 * HC * TPC], F8, tag="h2")
                ff2 = late.tile([128, 2 * FFC * TPC], F8, tag="ff")

                # ------------- Stage D: out_proj + residual + LN2 ------------
                with (
                    tc.tile_pool(name="atp", bufs=1) as atp,
                    tc.tile_pool(name="wop", bufs=4) as wop,
                    tc.tile_pool(name="lnD", bufs=4) as lnD,
                    tc.tile_pool(name="pso2", bufs=4, space="PSUM") as pso2_p,
                    tc.tile_pool(name="psstD", bufs=2, space="PSUM") as psstD,
                    tc.tile_pool(name="psbcD", bufs=2, space="PSUM") as psbcD,
                ):
                    at_sb = atp.tile([128, HC * TPC], F8, tag="at")
                    for j in range(HC):
                        buf = a2a_out_m[j % 2]
                        r = j // 2
                        nc.sync.dma_start(
                            at_sb[:, j * TPC:(j + 1) * TPC],
                            buf[r * 128:(r + 1) * 128, :])
                    for mo in range(HC):
                        ws = wop.tile([128, HID], F8, tag="wo")
                        nc.scalar.dma_start(ws[:], wo[mo * 128:(mo + 1) * 128, :])
                        ps = pso2_p.tile([128, TPC], F32, tag="o2")
                        for p in range(PAIRS):
                            nc.tensor.matmul(
                                ps[:],
                                _pair2(ws[:, p * 256:(p + 1) * 256]),
                                _pair2(at_sb[:, 2 * p * TPC:
                                             2 * (p + 1) * TPC]),
                                start=(p == 0), stop=(p == PAIRS - 1),
                                perf_mode=DR,
                            )
                        tt = lnD.tile([128, TPC], F32, tag="tt")
                        nc.scalar.activation(tt[:], ps[:], AF.Copy,
                                             scale=1.0 / (WS * AS))
                        xt = lnD.tile([128, TPC], F32, tag="xres")
                        nc.sync.dma_start(xt[:], xT[mo * 128:(mo + 1) * 128, :])
                        nc.vector.tensor_add(
                            x2_sb[:, mo * TPC:(mo + 1) * TPC], tt[:], xt[:])

                    # LN2 on x2 (f16 in SBUF)
                    ps_sx = psstD.tile([1, TPC], F32, tag="st")
                    ps_sq = psstD.tile([1, TPC], F32, tag="st")
                    for j in range(HC):
                        xj = x2_sb[:, j * TPC:(j + 1) * TPC]
                        sq = lnD.tile([128, TPC], F16, tag="sq")
                        nc.vector.tensor_mul(sq[:], xj, xj)
                        nc.tensor.matmul(ps_sx[:], ones_c16[:], xj,
                                         start=(j == 0), stop=(j == HC - 1))
                        nc.tensor.matmul(ps_sq[:], ones_c16[:], sq[:],
                                         start=(j == 0), stop=(j == HC - 1))
                    mu = lnD.tile([1, TPC], F32, tag="mu")
                    m2 = lnD.tile([1, TPC], F32, tag="m2")
                    var = lnD.tile([1, TPC], F32, tag="var")
                    lnv = lnD.tile([1, TPC], F32, tag="lnv")
                    rstd = lnD.tile([1, TPC], F16, tag="rstd")
                    mrs = lnD.tile([1, TPC], F32, tag="mrs")
                    mrs16 = lnD.tile([1, TPC], F16, tag="mrs16")
                    nc.vector.tensor_scalar_mul(mu[:], ps_sx[:], 1.0 / HID)
                    nc.vector.tensor_scalar_mul(m2[:], ps_sq[:], 1.0 / HID)
                    nc.vector.tensor_mul(var[:], mu[:], mu[:])
                    nc.vector.tensor_sub(var[:], m2[:], var[:])
                    nc.scalar.activation(lnv[:], var[:], AF.Ln, bias=eps_b[:])
                    nc.scalar.activation(rstd[:], lnv[:], AF.Exp,
                                         bias=zero1_b[:], scale=-0.5)
                    nc.vector.tensor_mul(mrs[:], mu[:], rstd[:])
                    nc.vector.tensor_scalar_mul(mrs16[:], mrs[:], -1.0)
                    ps_c1 = psbcD.tile([128, TPC], F32, tag="bc")
                    ps_c0 = psbcD.tile([128, TPC], F32, tag="bc")
                    nc.tensor.matmul(ps_c1[:], ones_r16[:], rstd[:],
                                     start=True, stop=True)
                    nc.tensor.matmul(ps_c0[:], ones_r16[:], mrs16[:],
                                     start=True, stop=True)
                    for j in range(HC):
                        t1 = lnD.tile([128, TPC], F32, tag="t1")
                        t2 = lnD.tile([128, TPC], F32, tag="t2")
                        nc.vector.tensor_mul(
                            t1[:], x2_sb[:, j * TPC:(j + 1) * TPC], ps_c1[:])
                        nc.vector.tensor_add(t2[:], t1[:], ps_c0[:])
                        h2t = lnD.tile([128, TPC], F16, tag="h2t")
                        nc.gpsimd.tensor_scalar(h2t[:], t2[:],
                                                g2_sb[:, j:j + 1],
                                                b2_sb[:, j:j + 1], MULT, ADD)
                        chi = j * TPC
                        clo = HC * TPC + j * TPC
                        nc.vector.tensor_copy(h2_2[:, chi:chi + TPC], h2t[:])
                        nc.vector.tensor_sub(h2_2[:, clo:clo + TPC], h2t[:],
                                             h2_2[:, chi:chi + TPC])

                # ------------- Stage E: MLP ----------------------------------
                with (
                    tc.tile_pool(name="wf1p", bufs=4) as wf1p,
                    tc.tile_pool(name="wf2p", bufs=4) as wf2p,
                    tc.tile_pool(name="outp", bufs=3) as outp,
                    tc.tile_pool(name="psf1", bufs=4, space="PSUM") as psf1_p,
                    tc.tile_pool(name="psf2", bufs=4, space="PSUM") as psf2_p,
                ):
                    for mo in range(FFC):
                        ws = wf1p.tile([128, 2 * HID], F8, tag="wf1")
                        nc.scalar.dma_start(ws[:],
                                            wf1[mo * 128:(mo + 1) * 128, :])
                        ps = psf1_p.tile([128, TPC], F32, tag="f1")
                        n3 = 3 * PAIRS
                        i = 0
                        for (wb, ab) in ((0, 0), (HID, 0), (0, HC * TPC)):
                            for p in range(PAIRS):
                                nc.tensor.matmul(
                                    ps[:],
                                    _pair2(ws[:, wb + p * 256:
                                              wb + (p + 1) * 256]),
                                    _pair2(h2_2[:, ab + 2 * p * TPC:
                                                ab + 2 * (p + 1) * TPC]),
                                    start=(i == 0), stop=(i == n3 - 1),
                                    perf_mode=DR,
                                )
                                i += 1
                        fft = outp.tile([128, TPC], F16, tag="fft")
                        nc.scalar.activation(fft[:], ps[:], AF.Gelu,
                                             bias=zero_b[:], scale=1.0 / WS)
                        chi = mo * TPC
                        clo = FFC * TPC + mo * TPC
                        nc.vector.tensor_copy(ff2[:, chi:chi + TPC], fft[:])
                        nc.vector.tensor_sub(ff2[:, clo:clo + TPC], fft[:],
                                             ff2[:, chi:chi + TPC])
                    for mo in range(HC):
                        ws = wf2p.tile([128, 2 * FFN], F8, tag="wf2")
                        nc.scalar.dma_start(ws[:],
                                            wf2[mo * 128:(mo + 1) * 128, :])
                        ps = psf2_p.tile([128, TPC], F32, tag="f2")
                        n3 = 3 * FPAIRS
                        i = 0
                        for (wb, ab) in ((0, 0), (FFN, 0), (0, FFC * TPC)):
                            for p in range(FPAIRS):
                                nc.tensor.matmul(
                                    ps[:],
                                    _pair2(ws[:, wb + p * 256:
                                              wb + (p + 1) * 256]),
                                    _pair2(ff2[:, ab + 2 * p * TPC:
                                               ab + 2 * (p + 1) * TPC]),
                                    start=(i == 0), stop=(i == n3 - 1),
                                    perf_mode=DR,
                                )
                                i += 1
                        tt = outp.tile([128, TPC], F32, tag="tt")
                        nc.scalar.activation(tt[:], ps[:], AF.Copy,
                                             scale=1.0 / WS)
                        ot = outp.tile([128, TPC], F32, tag="ot")
                        nc.vector.tensor_add(
                            ot[:], tt[:], x2_sb[:, mo * TPC:(mo + 1) * TPC])
                        nc.sync.dma_start(outT[mo * 128:(mo + 1) * 128, :],
                                          ot[:])
    return nc


def _build():
    if "nc" in _CACHE:
        return _CACHE["nc"]
    nc = bacc.Bacc(
        "TRN2", target_bir_lowering=False, debug=False,
        enable_asserts=True, num_devices=NCORES,
    )
    _emit(nc)
    nc.compile()
    _CACHE["nc"] = nc
    return nc


def _q8(a):
    import ml_dtypes
    return np.ascontiguousarray(a).astype(ml_dtypes.float8_e4m3)


def _q8_2l(a):
    import ml_dtypes
    hi = a.astype(ml_dtypes.float8_e4m3)
    lo = (a - hi.astype(np.float32)).astype(ml_dtypes.float8_e4m3)
    return hi, lo


def _wlay_qk(rows):
    """q/k weight strips [NH*128, HID] -> [128, PAIRS*NH*2*128] fp8 lhsT
    layout: [kpart][pair][m][plane][128out]."""
    t = rows.T.astype(np.float32)                  # [HID, NH*128]
    t = t.reshape(PAIRS, 2, 128, NH, 128)          # [p, pl, kpart, m, out]
    t = t.transpose(2, 0, 3, 1, 4)                 # [kpart, p, m, pl, out]
    return t.reshape(128, PAIRS * NH * 2 * 128)


def _wlay_v(rows):
    """v weights [NH*128, HID] -> [128, PAIRS*2*NH*128] moving layout:
    [kpart][pair][plane][NH*128]."""
    t = rows.T.astype(np.float32)                  # [HID, NH*128]
    t = t.reshape(PAIRS, 2, 128, NH * 128)         # [p, pl, kpart, col]
    t = t.transpose(2, 0, 1, 3)                    # [kpart, p, pl, col]
    return t.reshape(128, PAIRS * 2 * NH * 128)


def _wlay_out(W, n_strips, n_pairs):
    """W [n_strips*128, K] -> [n_strips*128, K] fp8 lhsT strips with
    [kpart][pair][plane][128out] per-strip layout."""
    K = W.shape[1]
    t = W.reshape(n_strips, 128, n_pairs, 2, 128)  # [mo, out, p, pl, kpart]
    t = t.transpose(0, 4, 2, 3, 1)                 # [mo, kpart, p, pl, out]
    return t.reshape(n_strips * 128, K)


def prepare_inputs(x, pe, w_qkv, w_out, w_fc1, w_fc2, g1, b1, g2, b2):
    x = np.asarray(x, np.float32)
    pe = np.asarray(pe, np.float32)
    w_qkv = np.asarray(w_qkv, np.float32) * WS
    w_out = np.asarray(w_out, np.float32) * WS
    w_fc1 = np.asarray(w_fc1, np.float32) * WS
    w_fc2 = np.asarray(w_fc2, np.float32) * WS

    xf = x.reshape(TOK, HID)
    perm = np.r_[np.arange(0, 128, 2), np.arange(1, 128, 2)]

    ropeC = np.tile(pe[:, 0::2].T, (2, B)).astype(np.float16)   # [128, TOK]
    ropeS = np.tile(pe[:, 1::2].T, (2, B)).astype(np.float16)

    gb = [np.asarray(v, np.float32).reshape(HC, 128).T.copy()
          for v in (g1, b1, g2, b2)]

    wo_hi = _q8(_wlay_out(w_out, HC, PAIRS))
    f1_hi, f1_lo = _q8_2l(_wlay_out(w_fc1, FFC, PAIRS))
    wf1_h = np.concatenate([f1_hi, f1_lo], axis=1)        # [FFC*128, 2*HID]
    f2_hi, f2_lo = _q8_2l(_wlay_out(w_fc2, HC, FPAIRS))
    wf2_h = np.concatenate([f2_hi, f2_lo], axis=1)        # [HC*128, 2*FFN]

    in_maps = []
    for c in range(NCORES):
        heads = [NH * c + i for i in range(NH)]
        qrows = np.concatenate([w_qkv[h * D + perm] for h in heads])
        krows = np.concatenate([w_qkv[HID + h * D + perm] for h in heads])
        vrows = np.concatenate([w_qkv[2 * HID + h * D: 2 * HID + (h + 1) * D]
                                for h in heads])
        xTc = np.ascontiguousarray(xf[c * TPC:(c + 1) * TPC].T)  # [HID, TPC]
        in_maps.append({
            "xT": xTc,
            "wq": _q8(_wlay_qk(qrows)), "wk": _q8(_wlay_qk(krows)),
            "wv": _q8(_wlay_v(vrows)),
            "wo": wo_hi, "wf1": wf1_h, "wf2": wf2_h,
            "g1": gb[0], "b1": gb[1], "g2": gb[2], "b2": gb[3],
            "ropeC": ropeC, "ropeS": ropeS,
        })
    return in_maps


def run(in_maps, **kwargs):
    nc = _build()
    return bass_utils.run_bass_kernel_spmd(
        nc, in_maps, core_ids=list(range(NCORES)), **kwargs
    )


def kernel(x, pe, w_qkv, w_out, w_fc1, w_fc2, g1, b1, g2, b2):
    in_maps = prepare_inputs(x, pe, w_qkv, w_out, w_fc1, w_fc2, g1, b1, g2, b2)
    res = run(in_maps)
    fullT = np.concatenate([res.results[c]["outT"] for c in range(NCORES)],
                           axis=1)
    return np.ascontiguousarray(fullT.T).reshape(B, S, HID).astype(np.float32)
